# revision 19
# baseline (speedup 1.0000x reference)
"""Trainium2 Bass kernel for nn_BDHLayer (sparse attention layer).

Sharding: B x NH = 2 x 4 = 8 (b, h) pairs, one per NeuronCore.  Everything is
per-(b,h) independent except the decoder GEMM, whose partial (T, D) products
are AllReduce-summed across the 4 cores sharing a batch.

Device layout trick: the N axis is permuted (even indices then odd indices) so
RoPE pair partners live in separate contiguous halves.  The permutation is
applied host-side to encoder/encoder_v columns, state/decoder rows; it cancels
inside every contraction over N and is inverted host-side on new_state.
"""
import math
import sys

sys.path.insert(0, "/opt/trn_rl_repo")

import numpy as np
import ml_dtypes

import bass_rust
import concourse.bass as bass
import concourse.mybir as mybir
import concourse.tile as tile
from concourse.vector_clock import ScopedClock, VectorClock
from concourse.bass_utils import run_bass_kernel_spmd

BF16 = mybir.dt.bfloat16
F32 = mybir.dt.float32
ALU = mybir.AluOpType
ACTF = mybir.ActivationFunctionType

B, NH, T, D, N = 2, 4, 512, 256, 8192
NHALF = N // 2
THETA = 2.0**16
LN_EPS = 1e-5
SCALE = float(N) ** -0.5
TT_ = T // 128          # 4 t-tiles
NSL = 16                # n'-slices of 512
NPAIR = 8               # slice pairs (k, k+8)

N_CORES = 8

_nop_counter = [0]


class PatchedTileContext(tile.TileContext):
    """This container's walrus rejects instructions with >1 sync-wait.
    Split extra waits onto InstNoOp instructions on the same engine, and chunk
    the final drain into one drain per outstanding proc."""

    def _split_waits(self, inst):
        si = inst.sync_info
        if si is None or not si.on_wait or len(si.on_wait) <= 1:
            return
        waits = list(si.on_wait)
        for w in waits[:-1]:
            _nop_counter[0] += 1
            nop = mybir.InstNoOp(name=f"waitnop_{_nop_counter[0]}", ins=[], outs=[])
            nop.engine = inst.engine
            nop.sync_info = bass_rust.SyncInfo(on_wait=[w], on_update=[])
            self._add_instruction(nop)
        si.on_wait = [waits[-1]]

    def _commit_and_lower(self, inst, bb, old_bb_map, bb_to_exit_bb):
        self._split_waits(inst)
        return super()._commit_and_lower(inst, bb, old_bb_map, bb_to_exit_bb)

    def _commit_instruction(self, inst, lazy_reg_writes=True):
        self._split_waits(inst)
        return super()._commit_instruction(inst, lazy_reg_writes=lazy_reg_writes)

    def _drain_and_barrier(self, tick_clock, wait_clock):
        gc = tick_clock.global_clock
        n = len(gc)
        for i in range(n):
            if gc[i] <= 0:
                continue
            vec = [gc[j] if j == i else 0 for j in range(n)]
            d = self.nc.sync.drain(fusable=False)
            wait_clock.add_sem_waits(d.ins, ScopedClock({None: VectorClock(vec)}))
        self.nc.all_engine_barrier()
        popped = self.nc._tile_sem_poison_stack.pop()
        assert popped is self._sem_poison
        self.nc.clear_and_free_semaphores(list(self.sems.allocated().values()))
        self.nc.all_engine_barrier()


def build_nc(debug=False):
    nc = bass.Bass("TRN2", target_bir_lowering=False, debug=False)

    di = lambda name, shape, dt: nc.dram_tensor(name, shape, dt, kind="ExternalInput").ap()
    do = lambda name, shape, dt: nc.dram_tensor(name, shape, dt, kind="ExternalOutput").ap()

    tensors = dict(
        enc=di("enc", [D, N], BF16),        # permuted columns, head slice
        encv=di("encv", [D, N], BF16),
        dec=di("dec", [N, D], BF16),        # permuted rows, head slice
        state=di("state", [N, D], F32),     # permuted rows
        state_bf=di("state_bf", [N, D], BF16),
        xs_in=di("xs", [T, D], F32),
        xsT_in=di("xsT", [D, T], F32),
        csT=di("csT", [NHALF, 2 * T], BF16),   # [cos | sin] (freq, t)
        csF=di("csF", [T, 2 * NHALF], BF16),   # [cos | sin] (t, freq)
        lnw=di("lnw", [128, D], F32),
        lnb=di("lnb", [128, D], F32),
        umask=di("umask", [128, 128], BF16),
        ident=di("ident", [128, 128], BF16),
        out_o=do("out_o", [T, D], F32),
        nstate_o=do("nstate_o", [N, D], F32),  # permuted rows
    )
    dbg = {}
    if debug:
        keys = debug if isinstance(debug, (set, list, tuple)) else \
            {"qrt", "xspt", "scores", "output", "ymlp"}
        if "qrt" in keys:
            dbg["qrt"] = do("dbg_qrt", [N, T], BF16)
        if "xspt" in keys:
            dbg["xspt"] = do("dbg_xspt", [N, T], BF16)
        if "scores" in keys:
            dbg["scores"] = do("dbg_scores", [T, T], F32)
        if "output" in keys:
            dbg["output"] = do("dbg_output", [T, D], F32)
        if "ymlp" in keys:
            dbg["ymlp"] = do("dbg_ymlp", [T, D], F32)
        if "xtn" in keys:
            dbg["xtn"] = do("dbg_xtn", [T, N], BF16)
        if "qrf" in keys:
            dbg["qrf"] = do("dbg_qrf", [T, N], BF16)
        if "chunk" in keys:
            dbg["chunk"] = do("dbg_chunk", [N, D], F32)
            dbg["stf"] = do("dbg_stf", [N, D], F32)
        if "xsb" in keys:
            dbg["xsb"] = do("dbg_xsb", [T, D], BF16)
            dbg["xsf"] = do("dbg_xsf", [T, D], F32)
            dbg["xstb"] = do("dbg_xstb", [D, T], BF16)

    with PatchedTileContext(nc) as tc:
        emit(nc, tc, tensors, dbg)
    return nc


def emit(nc, tc, tn, dbg):
    enc, encv, dec, state = tn["enc"], tn["encv"], tn["dec"], tn["state"]
    state_bf = tn["state_bf"]
    xs_in, xsT_in = tn["xs_in"], tn["xsT_in"]
    csT, csF = tn["csT"], tn["csF"]
    lnw, lnb, umask, ident = tn["lnw"], tn["lnb"], tn["umask"], tn["ident"]
    out_o, nstate_o = tn["out_o"], tn["nstate_o"]

    from contextlib import ExitStack

    est = ExitStack()
    consts = est.enter_context(tc.tile_pool(name="consts", bufs=1))
    dram = est.enter_context(tc.tile_pool(name="dram", bufs=1, space="DRAM"))

    # ---- constants ----
    xsT_f4 = consts.tile([128, 2 * T], F32, tag="xsTf4", name="xsTf4")
    xsT_b4 = consts.tile([128, 2 * T], BF16, tag="xsTb4", name="xsTb4")
    xs_f4 = consts.tile([128, TT_ * D], F32, tag="xsf4", name="xsf4")
    xs_b4 = consts.tile([128, TT_ * D], BF16, tag="xsb4", name="xsb4")
    xsT_f = [xsT_f4[:, i * T:(i + 1) * T] for i in range(2)]
    xsT_b = [xsT_b4[:, i * T:(i + 1) * T] for i in range(2)]
    xs_f = [xs_f4[:, i * D:(i + 1) * D] for i in range(TT_)]
    xs_b = [xs_b4[:, i * D:(i + 1) * D] for i in range(TT_)]
    lnw_t = consts.tile([128, D], F32, tag="lnw", name="lnw")
    lnb_t = consts.tile([128, D], F32, tag="lnb", name="lnb")
    um_t = consts.tile([128, 128], BF16, tag="um", name="um")
    id_t = consts.tile([128, 128], BF16, tag="id", name="id")
    eps_t = consts.tile([128, 1], F32, tag="eps", name="eps")
    nc.vector.memset(eps_t[:], LN_EPS)

    nc.sync.dma_start(xsT_f4[:].rearrange("p (a n) -> p a n", a=2),
                      xsT_in[:, :].rearrange("(a p) n -> p a n", p=128))
    nc.vector.tensor_copy(xsT_b4[:], xsT_f4[:])
    nc.sync.dma_start(xs_f4[:].rearrange("p (a n) -> p a n", a=TT_),
                      xs_in[:, :].rearrange("(a p) n -> p a n", p=128))
    nc.vector.tensor_copy(xs_b4[:], xs_f4[:])
    nc.sync.dma_start(lnw_t[:], lnw[:])
    nc.sync.dma_start(lnb_t[:], lnb[:])
    nc.sync.dma_start(um_t[:], umask[:])
    nc.sync.dma_start(id_t[:], ident[:])

    if "xsb" in dbg:
        for i in range(TT_):
            nc.sync.dma_start(dbg["xsb"][i * 128:(i + 1) * 128, :], xs_b[i][:])
            nc.sync.dma_start(dbg["xsf"][i * 128:(i + 1) * 128, :], xs_f[i][:])
        for i in range(2):
            nc.sync.dma_start(dbg["xstb"][i * 128:(i + 1) * 128, :], xsT_b[i][:])

    ar_in = dram.tile([T, D], F32, tag="arin", name="arin")
    ar_out = dram.tile([T, D], F32, tag="arout", name="arout")


    with tc.tile_pool(name="xsp", bufs=64) as xsp_p, \
         tc.tile_pool(name="ykv", bufs=1) as ykv_p:
        xspt = [xsp_p.tile([128, T], BF16, tag="xsp", name="xsp") for _ in range(64)]
        ykv = [ykv_p.tile([128, D], BF16, tag=f"ykv{i}", name=f"ykv{i}")
               for i in range(TT_)]
        ykvT = [ykv_p.tile([128, T], BF16, tag=f"ykvT{i}", name=f"ykvT{i}")
                for i in range(2)]

        with tc.tile_pool(name="qrt", bufs=64) as qrt_p, \
             tc.tile_pool(name="sco", bufs=4) as sco_p:
            qrt = [qrt_p.tile([128, T], BF16, tag="qrt", name="qrt") for _ in range(64)]
            scoT = [sco_p.tile([128, T], BF16, tag="sco", name="sco")
                    for _ in range(TT_)]

            # ---- phase B: x_latentT -> relu -> rope -> QRT ----
            with tc.tile_pool(name="encs", bufs=2) as encs, \
                 tc.tile_pool(name="tabs", bufs=2) as tabs, \
                 tc.tile_pool(name="ropet", bufs=3) as ropet, \
                 tc.tile_pool(name="psB", bufs=2, space="PSUM") as psB:
                for k in range(NPAIR):
                    for half in (0, 1):
                        sl = k + 8 * half
                        et4 = encs.tile([128, 1024], BF16, tag=f"e{half}",
                                        name=f"e{half}")
                        nc.sync.dma_start(
                            et4[:].rearrange("p (a n) -> p a n", a=2),
                            enc[:, sl * 512:(sl + 1) * 512]
                            .rearrange("(a p) n -> p a n", p=128))
                        for sub in range(4):
                            ps = psB.tile([128, 512], F32, tag="xlat", name="xlat")
                            for dt in range(2):
                                nc.tensor.matmul(
                                    ps[:],
                                    et4[:, dt * 512 + sub * 128:dt * 512 + (sub + 1) * 128],
                                    xsT_b[dt],
                                    start=(dt == 0), stop=(dt == 1))
                            nc.scalar.activation(xspt[sl * 4 + sub][:], ps[:],
                                                 ACTF.Relu)
                    cs4 = tabs.tile([128, 8 * T], BF16, tag="cs", name="cs")
                    nc.sync.dma_start(
                        cs4[:].rearrange("p (a n) -> p a n", a=4),
                        csT[k * 512:(k + 1) * 512, :]
                        .rearrange("(a p) n -> p a n", p=128))
                    for sub in range(4):
                        ct = cs4[:, sub * 2 * T:sub * 2 * T + T]
                        st = cs4[:, sub * 2 * T + T:(sub + 1) * 2 * T]
                        xe = xspt[k * 4 + sub]
                        xo = xspt[(k + 8) * 4 + sub]
                        qe = qrt[k * 4 + sub]
                        qo = qrt[(k + 8) * 4 + sub]
                        t1 = ropet.tile([128, 512], BF16, tag="t1", name="t1")
                        t2 = ropet.tile([128, 512], BF16, tag="t2", name="t2")
                        t3 = ropet.tile([128, 512], BF16, tag="t3", name="t3")
                        nc.vector.tensor_mul(t1[:], xe[:], ct)
                        nc.vector.tensor_mul(t2[:], xo[:], st)
                        nc.vector.tensor_sub(qe[:], t1[:], t2[:])
                        nc.gpsimd.tensor_mul(t3[:], xo[:], ct)
                        nc.vector.tensor_mul(t2[:], xe[:], st)
                        nc.gpsimd.tensor_add(qo[:], t3[:], t2[:])

            if "qrt" in dbg:
                for i in range(64):
                    nc.sync.dma_start(dbg["qrt"][i * 128:(i + 1) * 128, :], qrt[i][:])
            if "xspt" in dbg:
                for i in range(64):
                    nc.sync.dma_start(dbg["xspt"][i * 128:(i + 1) * 128, :], xspt[i][:])

            # ---- phase C: scoresT (s, t), strict upper ----
            with tc.tile_pool(name="psC", bufs=2, space="PSUM") as psC:
                for si in range(TT_):
                    t0 = si * 128
                    tlen = T - t0
                    ps = psC.tile([128, tlen], F32, tag="sc", name="sc")
                    for nt in range(64):
                        nc.tensor.matmul(ps[:], qrt[nt][:, t0:t0 + 128],
                                         qrt[nt][:, t0:T],
                                         start=(nt == 0), stop=(nt == 63))
                    nc.vector.scalar_tensor_tensor(scoT[si][:, t0:t0 + 128],
                                                   ps[:, 0:128], SCALE, um_t[:],
                                                   op0=ALU.mult, op1=ALU.mult)
                    if tlen > 128:
                        nc.scalar.activation(scoT[si][:, t0 + 128:T],
                                             ps[:, 128:tlen], ACTF.Copy, scale=SCALE)

            if "scores" in dbg:
                with tc.tile_pool(name="dbgzp", bufs=1) as dbgzp:
                    for si in range(TT_):
                        z = dbgzp.tile([128, T], F32, tag="dbgz", name="dbgz")
                        nc.vector.memset(z[:], 0.0)
                        nc.vector.tensor_copy(z[:, si * 128:T], scoT[si][:, si * 128:T])
                        nc.sync.dma_start(dbg["scores"][si * 128:(si + 1) * 128, :],
                                          z[:])

            # ---- phase D: output (t, d); LN -> yKV; transpose -> yKVT ----
            with tc.tile_pool(name="stb", bufs=3) as stb, \
                 tc.tile_pool(name="psD", bufs=4, space="PSUM") as psD, \
                 tc.tile_pool(name="lnt", bufs=2) as lnt, \
                 tc.tile_pool(name="psE", bufs=2, space="PSUM") as psE:
                psO = [psD.tile([128, D], F32, tag="o", name="o") for _ in range(TT_)]
                for g in range(16):
                    sb4 = stb.tile([128, 4 * D], BF16, tag="sb", name="sb")
                    nc.sync.dma_start(
                        sb4[:].rearrange("p (a n) -> p a n", a=4),
                        state_bf[g * 512:(g + 1) * 512, :]
                        .rearrange("(a p) n -> p a n", p=128))
                    for a in range(4):
                        nt = g * 4 + a
                        for tj in range(TT_):
                            nc.tensor.matmul(psO[tj][:],
                                             qrt[nt][:, tj * 128:(tj + 1) * 128],
                                             sb4[:, a * D:(a + 1) * D],
                                             start=(nt == 0), stop=False)
                for tj in range(TT_):
                    for si in range(tj + 1):
                        nc.tensor.matmul(psO[tj][:],
                                         scoT[si][:, tj * 128:(tj + 1) * 128],
                                         xs_b[si], start=False, stop=(si == tj))

                if "output" in dbg:
                    for tj in range(TT_):
                        zz = lnt.tile([128, D], F32, tag="dbgo", name="dbgo")
                        nc.vector.tensor_copy(zz[:], psO[tj][:])
                        nc.sync.dma_start(dbg["output"][tj * 128:(tj + 1) * 128, :],
                                          zz[:])

                for tj in range(TT_):
                    _ln_tile(nc, lnt, psO[tj], ykv[tj], lnw_t, lnb_t, eps_t)
                for dt in range(2):
                    for tj in range(TT_):
                        pt = psE.tile([128, 128], BF16, tag="tp", name="tp")
                        nc.tensor.transpose(pt[:],
                                            ykv[tj][:, dt * 128:(dt + 1) * 128],
                                            id_t[:])
                        nc.vector.tensor_copy(ykvT[dt][:, tj * 128:(tj + 1) * 128],
                                              pt[:])
        # qrt + sco pools closed here

        # ---- phase G: y path + decoder partial GEMM ----
        with tc.tile_pool(name="encv_s", bufs=2) as encv_s, \
             tc.tile_pool(name="dec_s", bufs=4) as dec_s, \
             tc.tile_pool(name="ysp", bufs=3) as ysp, \
             tc.tile_pool(name="xy", bufs=3) as xy, \
             tc.tile_pool(name="psG", bufs=2, space="PSUM") as psG, \
             tc.tile_pool(name="psM", bufs=4, space="PSUM") as psM, \
             tc.tile_pool(name="ymls", bufs=4) as ymls:
            psY = [psM.tile([128, D], F32, tag="ym", name="ym") for _ in range(TT_)]
            for sl in range(NSL):
                ev4 = encv_s.tile([128, 1024], BF16, tag="ev", name="ev")
                nc.sync.dma_start(
                    ev4[:].rearrange("p (a n) -> p a n", a=2),
                    encv[:, sl * 512:(sl + 1) * 512]
                    .rearrange("(a p) n -> p a n", p=128))
                db4 = dec_s.tile([128, 4 * D], BF16, tag="db", name="db")
                nc.sync.dma_start(
                    db4[:].rearrange("p (a n) -> p a n", a=4),
                    dec[sl * 512:(sl + 1) * 512, :]
                    .rearrange("(a p) n -> p a n", p=128))
                for sub in range(4):
                    ps = psG.tile([128, 512], F32, tag="yl", name="yl")
                    for dt in range(2):
                        nc.tensor.matmul(
                            ps[:],
                            ev4[:, dt * 512 + sub * 128:dt * 512 + (sub + 1) * 128],
                            ykvT[dt][:], start=(dt == 0), stop=(dt == 1))
                    ys = ysp.tile([128, 512], BF16, tag="ys", name="ys")
                    nc.scalar.activation(ys[:], ps[:], ACTF.Relu)
                    xyt = xy.tile([128, 512], BF16, tag="xyt", name="xyt")
                    nc.vector.tensor_mul(xyt[:], xspt[sl * 4 + sub][:], ys[:])
                    first = (sl == 0 and sub == 0)
                    last = (sl == NSL - 1 and sub == 3)
                    for tj in range(TT_):
                        nc.tensor.matmul(psY[tj][:],
                                         xyt[:, tj * 128:(tj + 1) * 128],
                                         db4[:, sub * D:(sub + 1) * D],
                                         start=first, stop=last)
            for tj in range(TT_):
                ym = ymls.tile([128, D], F32, tag="yms", name="yms")
                nc.vector.tensor_copy(ym[:], psY[tj][:])
                if "ymlp" in dbg:
                    nc.sync.dma_start(dbg["ymlp"][tj * 128:(tj + 1) * 128, :], ym[:])
                nc.sync.dma_start(ar_in[tj * 128:(tj + 1) * 128, :], ym[:])
    # xsp + ykv pools closed here

    # AllReduce of yMLP partials within each batch group
    nc.gpsimd.collective_compute(
        "AllReduce", ALU.add,
        replica_groups=[[0, 1, 2, 3], [4, 5, 6, 7]],
        ins=[ar_in.opt()], outs=[ar_out.opt()],
    )

    # ---- phase F: chunk_state + new_state (overlaps AllReduce) ----
    with tc.tile_pool(name="encs2", bufs=2) as encs2, \
         tc.tile_pool(name="tabs2", bufs=2) as tabs2, \
         tc.tile_pool(name="xtn", bufs=5) as xtn, \
         tc.tile_pool(name="stf2", bufs=3) as stf2, \
         tc.tile_pool(name="rtt", bufs=2) as rtt, \
         tc.tile_pool(name="qtn", bufs=5) as qtn, \
         tc.tile_pool(name="nst", bufs=3) as nst, \
         tc.tile_pool(name="psF", bufs=3, space="PSUM") as psF, \
         tc.tile_pool(name="psF2", bufs=3, space="PSUM") as psF2:
        for k in range(NPAIR):
            xtn_t = [[None] * TT_ for _ in range(2)]
            for half in (0, 1):
                sl = k + 8 * half
                et4 = encs2.tile([128, 1024], BF16, tag=f"e2{half}",
                                 name=f"e2{half}")
                nc.scalar.dma_start(
                    et4[:].rearrange("p (a n) -> p a n", a=2),
                    enc[:, sl * 512:(sl + 1) * 512]
                    .rearrange("(a p) n -> p a n", p=128))
                for tj in range(TT_):
                    ps = psF.tile([128, 512], F32, tag="xl2", name="xl2")
                    for dt in range(2):
                        nc.tensor.matmul(ps[:],
                                         xsT_b[dt][:, tj * 128:(tj + 1) * 128],
                                         et4[:, dt * 512:(dt + 1) * 512],
                                         start=(dt == 0), stop=(dt == 1))
                    xt = xtn.tile([128, 512], BF16, tag=f"xtn{half}",
                                  name=f"xtn{half}")
                    nc.scalar.activation(xt[:], ps[:], ACTF.Relu)
                    xtn_t[half][tj] = xt
            qr_eo = [[None] * TT_ for _ in range(2)]
            cf4 = tabs2.tile([128, 4 * 512], BF16, tag="cf", name="cf")
            sf4_ = tabs2.tile([128, 4 * 512], BF16, tag="sf_", name="sf_")
            nc.scalar.dma_start(
                cf4[:].rearrange("p (a n) -> p a n", a=4),
                csF[:, k * 512:(k + 1) * 512]
                .rearrange("(a p) n -> p a n", p=128))
            nc.scalar.dma_start(
                sf4_[:].rearrange("p (a n) -> p a n", a=4),
                csF[:, NHALF + k * 512:NHALF + (k + 1) * 512]
                .rearrange("(a p) n -> p a n", p=128))
            for tj in range(TT_):
                cf = cf4[:, tj * 512:(tj + 1) * 512]
                sf_ = sf4_[:, tj * 512:(tj + 1) * 512]
                t1 = rtt.tile([128, 512], BF16, tag="rt1", name="rt1")
                t2 = rtt.tile([128, 512], BF16, tag="rt2", name="rt2")
                qe = qtn.tile([128, 512], BF16, tag="qe", name="qe")
                qo = qtn.tile([128, 512], BF16, tag="qo", name="qo")
                t3 = rtt.tile([128, 512], BF16, tag="rt3", name="rt3")
                nc.vector.tensor_mul(t1[:], xtn_t[0][tj][:], cf)
                nc.vector.tensor_mul(t2[:], xtn_t[1][tj][:], sf_)
                nc.vector.tensor_sub(qe[:], t1[:], t2[:])
                nc.gpsimd.tensor_mul(t3[:], xtn_t[1][tj][:], cf)
                nc.vector.tensor_mul(t2[:], xtn_t[0][tj][:], sf_)
                nc.gpsimd.tensor_add(qo[:], t3[:], t2[:])
                qr_eo[0][tj] = qe
                qr_eo[1][tj] = qo
            if "xtn" in dbg:
                for half in (0, 1):
                    sl = k + 8 * half
                    for tj in range(TT_):
                        nc.sync.dma_start(
                            dbg["xtn"][tj * 128:(tj + 1) * 128,
                                       sl * 512:(sl + 1) * 512],
                            xtn_t[half][tj][:])
            if "qrf" in dbg:
                for half in (0, 1):
                    sl = k + 8 * half
                    for tj in range(TT_):
                        nc.sync.dma_start(
                            dbg["qrf"][tj * 128:(tj + 1) * 128,
                                       sl * 512:(sl + 1) * 512],
                            qr_eo[half][tj][:])
            for half in (0, 1):
                sl = k + 8 * half
                sf4 = stf2.tile([128, 4 * D], BF16, tag="sf2", name="sf2")
                nc.scalar.dma_start(
                    sf4[:].rearrange("p (a n) -> p a n", a=4),
                    state_bf[sl * 512:(sl + 1) * 512, :]
                    .rearrange("(a p) n -> p a n", p=128))
                ns4 = nst.tile([128, 4 * D], F32, tag="ns", name="ns")
                for sub in range(4):
                    pc = psF2.tile([128, D], F32, tag="ch", name="ch")
                    for tj in range(TT_):
                        nc.tensor.matmul(pc[:],
                                         qr_eo[half][tj][:, sub * 128:(sub + 1) * 128],
                                         xs_b[tj], start=(tj == 0), stop=(tj == 3))
                    nc.vector.scalar_tensor_tensor(
                        ns4[:, sub * D:(sub + 1) * D], pc[:], SCALE,
                        sf4[:, sub * D:(sub + 1) * D],
                        op0=ALU.mult, op1=ALU.add)
                nc.sync.dma_start(
                    nstate_o[sl * 512:(sl + 1) * 512, :]
                    .rearrange("(a p) n -> p a n", p=128),
                    ns4[:].rearrange("p (a n) -> p a n", a=4))

    # ---- phase H: final LNs ----
    with tc.tile_pool(name="fin", bufs=2) as fin, \
         tc.tile_pool(name="lnt2", bufs=2) as lnt2:
        for tj in range(TT_):
            ar_s = fin.tile([128, D], F32, tag="ars", name="ars")
            nc.sync.dma_start(ar_s[:], ar_out[tj * 128:(tj + 1) * 128, :])
            y_t = fin.tile([128, D], F32, tag="yt", name="yt")
            _ln_tile(nc, lnt2, ar_s, y_t, lnw_t, lnb_t, eps_t)
            xr = fin.tile([128, D], F32, tag="xr", name="xr")
            nc.vector.tensor_add(xr[:], y_t[:], xs_f[tj])
            o_t = fin.tile([128, D], F32, tag="ot", name="ot")
            _ln_tile(nc, lnt2, xr, o_t, lnw_t, lnb_t, eps_t)
            nc.sync.dma_start(out_o[tj * 128:(tj + 1) * 128, :], o_t[:])

    est.close()


def _ln_tile(nc, lnpool, src, dst, lnw_t, lnb_t, eps_t):
    """LayerNorm over the free axis (D) of a (128, D) tile src -> dst.
    src may be PSUM or SBUF fp32; dst any dtype."""
    st6 = lnpool.tile([128, 6], F32, tag="st6", name="st6")
    st2 = lnpool.tile([128, 2], F32, tag="st2", name="st2")
    nc.vector.bn_stats(st6[:], src[:])
    nc.vector.bn_aggr(st2[:], st6[:])
    sd = lnpool.tile([128, 1], F32, tag="sd", name="sd")
    nc.scalar.activation(sd[:], st2[:, 1:2], ACTF.Sqrt, bias=eps_t[:])
    rstd = lnpool.tile([128, 1], F32, tag="rstd", name="rstd")
    nc.vector.reciprocal(rstd[:], sd[:])
    nrm = lnpool.tile([128, D], F32, tag="nrm", name="nrm")
    nc.vector.tensor_scalar(nrm[:], src[:], st2[:, 0:1], rstd[:],
                            op0=ALU.subtract, op1=ALU.mult)
    wv = lnpool.tile([128, D], F32, tag="wv", name="wv")
    nc.vector.tensor_mul(wv[:], nrm[:], lnw_t[:])
    nc.vector.tensor_add(dst[:], wv[:], lnb_t[:])


# ---------------------------------------------------------------------------
# host side
# ---------------------------------------------------------------------------

def _tables(pos_offset):
    i = np.arange(NHALF, dtype=np.float32)
    f = (1.0 / (np.float32(THETA) ** ((2.0 * i).astype(np.float32) / np.float32(N)))
         / np.float32(2.0 * math.pi)).astype(np.float32)
    t = np.arange(pos_offset, pos_offset + T, dtype=np.float32)
    ph = t[:, None] * f[None, :]
    ang = (ph % 1.0) * np.float32(2.0 * math.pi)
    return np.cos(ang).astype(np.float32), np.sin(ang).astype(np.float32)


def host_prep(x, state, encoder, encoder_v, decoder, ln_w, ln_b, pos_offset):
    bf = ml_dtypes.bfloat16
    perm = np.concatenate([np.arange(0, N, 2), np.arange(1, N, 2)])
    cosf, sinf = _tables(int(pos_offset))
    csF = np.concatenate([cosf, sinf], axis=1).astype(bf)
    csT_ = np.concatenate([np.ascontiguousarray(cosf.T),
                           np.ascontiguousarray(sinf.T)], axis=1).astype(bf)
    lnw_b = np.broadcast_to(np.asarray(ln_w, np.float32), (128, D)).copy()
    lnb_b = np.broadcast_to(np.asarray(ln_b, np.float32), (128, D)).copy()
    um = np.triu(np.ones((128, 128), np.float32), k=1).astype(bf)
    ident = np.eye(128, dtype=np.float32).astype(bf)

    in_maps = []
    for core in range(N_CORES):
        b, h = core // NH, core % NH
        xs = np.asarray(x[b, 0], np.float32)
        in_maps.append({
            "enc": np.ascontiguousarray(np.asarray(encoder[h], np.float32)[:, perm]).astype(bf),
            "encv": np.ascontiguousarray(np.asarray(encoder_v[h], np.float32)[:, perm]).astype(bf),
            "dec": np.ascontiguousarray(np.asarray(decoder[h * N:(h + 1) * N], np.float32)[perm]).astype(bf),
            "state": np.ascontiguousarray(np.asarray(state[b, h], np.float32)[perm]),
            "state_bf": np.ascontiguousarray(np.asarray(state[b, h], np.float32)[perm]).astype(bf),
            "xs": xs,
            "xsT": np.ascontiguousarray(xs.T),
            "csT": csT_, "csF": csF,
            "lnw": lnw_b, "lnb": lnb_b, "umask": um, "ident": ident,
        })
    return in_maps, perm


def gather(results, perm):
    invperm = np.empty(N, np.int64)
    invperm[perm] = np.arange(N)
    out = np.stack([results[0]["out_o"], results[NH]["out_o"]])[:, None]
    new_state = np.empty((B, NH, N, D), np.float32)
    for core in range(N_CORES):
        b, h = core // NH, core % NH
        new_state[b, h] = results[core]["nstate_o"][invperm]
    return out.astype(np.float32), new_state


_cached = {}


def kernel(x, state, encoder, encoder_v, decoder, ln_w, ln_b, pos_offset):
    in_maps, perm = host_prep(x, state, encoder, encoder_v, decoder, ln_w,
                              ln_b, pos_offset)
    if "nc" not in _cached:
        _cached["nc"] = build_nc(debug=False)
    nc = _cached["nc"]
    res = run_bass_kernel_spmd(nc, in_maps, core_ids=list(range(N_CORES)))
    return gather(res.results, perm)
